# revision 1
# baseline (speedup 1.0000x reference)
"""Trainium2 Bass kernel for nn_CGTensorProductEquivariantModel.

Self-contained: hardcodes all shapes. Accepts FULL inputs, returns FULL output.

Strategy (8 NeuronCores, SPMD):
  - Host sorts edges by src node, shards them so core c owns all edges whose
    src is in node range [c*1280, (c+1)*1280) (nodes padded 10000->10240).
  - Node range is split into 10 windows of 128 nodes; each window gets a fixed
    number of 128-edge tiles (T_w, global max, pad edges as needed) so all 8
    cores run the identical static program.
  - Per edge tile: fc1 (PE) -> relu (ACT) -> fc2 in col chunks (PE, weights
    permuted to (o,i) order with path norms folded in) -> PSUM->SBUF copy
    (ACT) -> per-edge TP via broadcast-AP multiplies (DVE) -> scatter-add to
    the window's node accumulator via one-hot matmuls (PE), with the ss/vs
    i-contraction fused into per-i accumulating matmuls.
  - Each core ends with complete node sums+counts for its own 1280 nodes: no
    collective needed for the segment sum. Mean/residual computed locally;
    batchnorm statistics via ones-matmul partials + one tiny AllReduce; final
    normalize locally; host concatenates the 8 output shards.
"""
import os
import sys
import math
import numpy as np

sys.path.insert(0, '/opt/trn_rl_repo')

import concourse.bass as bass            # noqa: E402
import concourse.bacc as bacc            # noqa: E402
import concourse.mybir as mybir          # noqa: E402
import concourse.tile as tile            # noqa: E402
from concourse import bass_utils         # noqa: E402

dt = mybir.dt
AF = mybir.ActivationFunctionType
ALU = mybir.AluOpType
AX = mybir.AxisListType

# ---- problem constants (hardcoded; must match reference.py) ----
NS, NV = 48, 10
N_NODES, N_EDGES = 10000, 100000
F = 3 * NS                       # 144 edge features / fc1 width
IN_DIM = NS + 3 * NV             # 78
EPS = 1e-5
CW = NS * NS + NV * NS + NS * NV + NV * NV   # 3364
OVS = NS * NS
OSV = OVS + NV * NS
OVV = OSV + NS * NV
NCORES = 8
WIN = 128                        # nodes per window
WPC = 10                         # windows per core
CPN = WIN * WPC                  # 1280 nodes per core
NODE_PAD = NCORES * CPN          # 10240
ET = 128                         # edges per tile

# dtypes for compute stages (perf knobs)
MM_DT = dt.float32               # fc1/fc2 matmul operand tag (float32r for speed)
SC_DT = dt.float32               # scatter matmul / products dtype

_cache = {}
BENCH_NO_COLLECTIVE = False   # replace AllReduce with local copy (TimelineSim)
IO_BF16 = True                # bf16 edge-stream inputs / matmuls / products
CFG = {
    'combine_eng': 'gpsimd',   # engine for svt*s1+u+tpu combines
    'tail_copy_dve': True,     # 292-col copy on DVE instead of ACT
    'ss_quarters': False,      # second ss pre-reduce level
    'ss_pre': True, 'vs_pre': False,  # enable pre-reduce stages
    'small_prod_eng': 'vector',  # vs/sv/vv products engine
    'io_bufs': 4, 'work_bufs': 4,
    'two_bank': False,  # split scatter accumulation across two psum banks
}

import ml_dtypes  # noqa: E402
NP_BF16 = np.dtype(ml_dtypes.bfloat16)


# ----------------------------------------------------------------------------
# Host-side preprocessing
# ----------------------------------------------------------------------------

def _prep(node_attr, edge_attr, edge_sh, fc1_w, fc1_b, fc2_w, fc2_b,
          bn_weight, bn_bias, edge_index):
    f32 = np.float32
    E = edge_attr.shape[0]
    src = np.asarray(edge_index[0]).astype(np.int64)
    dst = np.asarray(edge_index[1]).astype(np.int64)

    # per-edge host precomputes
    x = node_attr[dst].astype(f32)                    # [E, 78]
    xs = x[:, :NS]
    xv = x[:, NS:].reshape(E, NV, 3)
    s0 = edge_sh[:, 0].astype(f32)
    s1 = edge_sh[:, 1:4].astype(f32)
    xs1 = xs * s0[:, None]                            # s0 folded (ss block)
    dott = np.einsum('eic,ec->ei', xv, s1).astype(f32)   # [E, NV]
    xvc = (xv * s0[:, None, None]).transpose(0, 2, 1).reshape(E, 3 * NV)  # (c,i)

    # fc2 weight permutation to (o,i) + path-norm folding
    a_ss = 1.0 / math.sqrt(NS * 2.0)
    a_vs = 1.0 / math.sqrt(NV * 2.0) / math.sqrt(3.0)
    a_sv = 1.0 / math.sqrt(NS * 2.0)
    a_vv = 1.0 / math.sqrt(NV * 2.0)

    def perm(mat):
        w_ss = mat[:, :OVS].reshape(-1, NS, NS).transpose(0, 2, 1) * a_ss
        w_vs = mat[:, OVS:OSV].reshape(-1, NV, NS).transpose(0, 2, 1) * a_vs
        w_sv = mat[:, OSV:OVV].reshape(-1, NS, NV).transpose(0, 2, 1) * a_sv
        w_vv = mat[:, OVV:].reshape(-1, NV, NV).transpose(0, 2, 1) * a_vv
        return [w_ss.reshape(-1, NS * NS), w_vs.reshape(-1, NS * NV),
                w_sv.reshape(-1, NV * NS), w_vv.reshape(-1, NV * NV)]

    w2p = np.concatenate(perm(fc2_w.astype(f32)), axis=1)         # [F, CW]
    b_ss, b_vs, b_sv, b_vv = perm(fc2_b.astype(f32)[None, :])
    b_ss = b_ss.reshape(NS, NS)   # (o,i)
    b_vs = b_vs.reshape(NS, NV)
    b_sv = b_sv.reshape(NV, NS)
    b_vv = b_vv.reshape(NV, NV)

    # fc2_b contribution rides in via tpu (also carries the counts column)
    tpb_s = xs1 @ b_ss.T + dott @ b_vs.T              # [E, NS]
    sv_b = xs @ b_sv.T                                # [E, NV]
    u_b = np.einsum('oi,eci->eoc', b_vv, xvc.reshape(E, 3, NV))
    tpb_v = sv_b[:, :, None] * s1[:, None, :] + u_b   # [E, NV, 3]
    tpu = np.concatenate([tpb_s, tpb_v.reshape(E, 3 * NV),
                          np.ones((E, 1), f32)], axis=1)          # [E, 79]

    # window / tile assignment: sort windows by edge count, hand slot s the
    # s-th group of 8 so every core's slot-s window needs the same tile count
    g = src // WIN                                    # global window 0..79
    nwin = NCORES * WPC
    cnt_w = np.bincount(g, minlength=nwin)
    worder = np.argsort(-cnt_w, kind='stable')
    core_of_win = np.empty(nwin, np.int64)
    slot_of_win = np.empty(nwin, np.int64)
    t_list = []
    for s in range(WPC):
        grp = worder[s * NCORES:(s + 1) * NCORES]
        core_of_win[grp] = np.arange(NCORES)
        slot_of_win[grp] = s
        t_list.append(max(1, int(np.ceil(cnt_w[grp].max() / ET))))
    if sum(t_list) % 2:
        t_list[-1] += 1
    t_list = tuple(t_list)
    NT = sum(t_list)                                  # tiles per core
    Te = NT * ET                                      # edge slots per core
    sbase = np.concatenate([[0], np.cumsum(t_list)[:-1]]) * ET

    eorder = np.argsort(g, kind='stable')
    slot_of = np.empty(E, np.int64)                   # edge -> (core, slot)
    core_of = np.empty(E, np.int64)
    pos = 0
    for gw in range(nwin):
        n = cnt_w[gw]
        idx = eorder[pos:pos + n]
        pos += n
        slot_of[idx] = sbase[slot_of_win[gw]] + np.arange(n)
        core_of[idx] = core_of_win[gw]

    nid_rel = (src - g * WIN).astype(np.int64)        # 0..127 within window

    per_core = []
    for c in range(NCORES):
        m = core_of == c
        sl = slot_of[m]
        ea = np.zeros((Te, F), f32);   ea[sl] = edge_attr[m]
        a_xs1 = np.zeros((Te, NS), f32);  a_xs1[sl] = xs1[m]
        a_dot = np.zeros((Te, NV), f32);  a_dot[sl] = dott[m]
        a_xs = np.zeros((Te, NS), f32);   a_xs[sl] = xs[m]
        a_s1 = np.zeros((Te, 3), f32);    a_s1[sl] = s1[m]
        a_xvc = np.zeros((Te, 3 * NV), f32); a_xvc[sl] = xvc[m]
        a_tpu = np.zeros((Te, 79), f32);  a_tpu[sl] = tpu[m]
        a_oh = np.zeros((Te, WIN), f32)
        a_oh[sl, nid_rel[m]] = 1.0
        na = np.zeros((CPN, IN_DIM), f32)
        for gw in range(nwin):
            if core_of_win[gw] != c:
                continue
            s = slot_of_win[gw]
            lo, hi = gw * WIN, min((gw + 1) * WIN, N_NODES)
            if hi > lo:
                na[s * WIN:s * WIN + hi - lo] = node_attr[lo:hi]
        edata = np.concatenate(
            [a_xs1, a_xs, a_dot, a_xvc, a_tpu, a_s1, a_oh], axis=1)  # [Te, 346]
        ed3 = edata.reshape(NT, ET, -1)
        edata2 = np.concatenate([ed3[0::2], ed3[1::2]], axis=2)  # [NT/2, ET, 692]
        edata2 = np.ascontiguousarray(edata2.reshape(NT // 2 * ET, -1))
        entry = {'eaT': np.ascontiguousarray(ea.T), 'edata': edata2, 'na': na}
        if IO_BF16:
            entry['eaT'] = entry['eaT'].astype(NP_BF16)
            entry['edata'] = entry['edata'].astype(NP_BF16)
        per_core.append(entry)

    consts = {
        'fc1w': fc1_w.astype(NP_BF16 if IO_BF16 else f32),  # [F, F] (K=f rows)
        'fc1b': fc1_b.astype(f32).reshape(F, 1),
        'w2p': w2p.astype(NP_BF16 if IO_BF16 else f32),     # [F, CW]
        'bnw_s': bn_weight[:NS].astype(f32).reshape(1, NS),
        'bnw_v': bn_weight[NS:].astype(f32).reshape(1, NV),
        'bnb_s': bn_bias.astype(f32).reshape(1, NS),
    }
    return per_core, consts, t_list, (core_of_win, slot_of_win)


# ----------------------------------------------------------------------------
# Device program
# ----------------------------------------------------------------------------

def CFG_ACC_BUFS():
    return 1 if CFG['two_bank'] else 2


def _build(t_list):
    NT = sum(t_list)
    Te = NT * ET
    tile_meta = []                                    # (slot, first, last)
    for s, tw in enumerate(t_list):
        for t in range(tw):
            tile_meta.append((s, t == 0, t == tw - 1))
    fp = dt.float32
    bfd = dt.bfloat16 if IO_BF16 else fp
    under_axon = bass_utils.axon_active()
    nc = bacc.Bacc('TRN2', target_bir_lowering=False, debug=not under_axon,
                   enable_asserts=True, num_devices=NCORES)

    # I/O
    d_eaT = nc.dram_tensor('eaT', [F, Te], bfd, kind='ExternalInput').ap()
    ED = 2 * NS + NV + 3 * NV + 79 + 3 + WIN   # 346 packed per-edge cols
    d_edata = nc.dram_tensor('edata', [NT // 2 * ET, 2 * ED], bfd,
                             kind='ExternalInput').ap()
    d_na = nc.dram_tensor('na', [CPN, IN_DIM], fp, kind='ExternalInput').ap()
    d_fc1w = nc.dram_tensor('fc1w', [F, F], bfd, kind='ExternalInput').ap()
    d_fc1b = nc.dram_tensor('fc1b', [F, 1], fp, kind='ExternalInput').ap()
    d_w2p = nc.dram_tensor('w2p', [F, CW], bfd, kind='ExternalInput').ap()
    d_bnw_s = nc.dram_tensor('bnw_s', [1, NS], fp, kind='ExternalInput').ap()
    d_bnw_v = nc.dram_tensor('bnw_v', [1, NV], fp, kind='ExternalInput').ap()
    d_bnb_s = nc.dram_tensor('bnb_s', [1, NS], fp, kind='ExternalInput').ap()
    d_out = nc.dram_tensor('out_shard', [CPN, IN_DIM], fp,
                           kind='ExternalOutput').ap()

    with tile.TileContext(nc) as tc:
        with tc.tile_pool(name='const', bufs=1) as cpool, \
             tc.tile_pool(name='persist', bufs=1) as ppool:
            fc1w_hi = cpool.tile([128, F], bfd)
            nc.sync.dma_start(fc1w_hi[:], d_fc1w[0:128, :])
            fc1w_lo = cpool.tile([16, F], bfd)
            nc.sync.dma_start(fc1w_lo[:], d_fc1w[128:F, :])
            fc1b_hi = cpool.tile([128, 1], fp)
            nc.sync.dma_start(fc1b_hi[:], d_fc1b[0:128, :])
            fc1b_lo = cpool.tile([16, 1], fp)
            nc.sync.dma_start(fc1b_lo[:], d_fc1b[128:F, :])
            w2_hi = cpool.tile([128, CW], bfd)
            nc.sync.dma_start(w2_hi[:], d_w2p[0:128, :])
            w2_lo = cpool.tile([16, CW], bfd)
            nc.sync.dma_start(w2_lo[:], d_w2p[128:F, :])
            bnw_s = cpool.tile([1, NS], fp); nc.sync.dma_start(bnw_s[:], d_bnw_s[:])
            bnw_v = cpool.tile([1, NV], fp); nc.sync.dma_start(bnw_v[:], d_bnw_v[:])
            bnb_s = cpool.tile([1, NS], fp); nc.sync.dma_start(bnb_s[:], d_bnb_s[:])
            ones128 = cpool.tile([128, 1], fp); nc.vector.memset(ones128[:], 1.0)
            ones1 = cpool.tile([1, 128], fp); nc.vector.memset(ones1[:], 1.0)
            epsc = cpool.tile([1, 1], fp); nc.vector.memset(epsc[:], EPS)

            sums_sb = ppool.tile([128, WPC * 79], fp)
            resid_sb = ppool.tile([128, WPC * IN_DIM], fp)
            na_sb = ppool.tile([128, WPC * IN_DIM], fp)
            for w in range(WPC):
                nc.sync.dma_start(na_sb[:, w * IN_DIM:(w + 1) * IN_DIM],
                                  d_na[w * 128:(w + 1) * 128, :])

            # ---------------- phase 1: edge tiles ----------------
            with tc.tile_pool(name='io', bufs=CFG['io_bufs']) as io, \
                 tc.tile_pool(name='work', bufs=CFG['work_bufs']) as work, \
                 tc.tile_pool(name='ps_h', bufs=2, space='PSUM') as ps_h, \
                 tc.tile_pool(name='ps_w', bufs=2, space='PSUM') as ps_w, \
                 tc.tile_pool(name='ps_acc', bufs=CFG_ACC_BUFS(), space='PSUM') as ps_acc, \
                 tc.tile_pool(name='ps_accb', bufs=CFG_ACC_BUFS(), space='PSUM') as ps_accb:
                acc = None
                for ti in range(NT):
                    w, first, last = tile_meta[ti]
                    e0 = ti * ET
                    if first:
                        acc = ps_acc.tile([WIN, 79], fp, tag='acc', name='acc')
                        accb = (ps_accb.tile([WIN, 48], fp, tag='accb', name='accb')
                                if CFG['two_bank'] else None)
                    if ti % 2 == 0:
                        p0 = (ti // 2) * ET
                        eaT2_hi = io.tile([128, 2 * ET], bfd, tag='eaT_hi')
                        nc.sync.dma_start(eaT2_hi[:], d_eaT[0:128, e0:e0 + 2 * ET])
                        eaT2_lo = io.tile([16, 2 * ET], bfd, tag='eaT_lo')
                        nc.sync.dma_start(eaT2_lo[:], d_eaT[128:F, e0:e0 + 2 * ET])
                        edt2 = io.tile([ET, 2 * ED], bfd, tag='edata')
                        nc.sync.dma_start(edt2[:], d_edata[p0:p0 + ET, :])
                    sub = ti % 2
                    eaT_hi = eaT2_hi[:, sub * ET:(sub + 1) * ET]
                    eaT_lo = eaT2_lo[:, sub * ET:(sub + 1) * ET]
                    edt = edt2[:, sub * ED:(sub + 1) * ED]
                    if True:
                        xs1 = edt[:, 0:NS]
                        xst = edt[:, NS:2 * NS]
                        dott = edt[:, 2 * NS:2 * NS + NV]
                        xvc = edt[:, 106:136]
                        tpu = edt[:, 136:215]
                        s1t = edt[:, 215:218]
                        oh = edt[:, 218:218 + WIN]

                        # fc1 -> hT (both chunks share one psum bank)
                        hps = ps_h.tile([128, 2 * ET], fp, tag='hps', name='hps')
                        hA = hps[:, 0:ET]
                        hB = hps[0:16, ET:2 * ET]
                        m1 = nc.tensor.matmul(hA, fc1w_hi[:, 0:128], eaT_hi,
                                              start=True, stop=False)
                        nc.tensor.matmul(hA, fc1w_lo[:, 0:128], eaT_lo,
                                         start=False, stop=True)
                        m3 = nc.tensor.matmul(hB, fc1w_hi[:, 128:F], eaT_hi,
                                              start=False, stop=False,
                                              skip_group_check=True)
                        nc.tensor.matmul(hB, fc1w_lo[:, 128:F], eaT_lo,
                                         start=False, stop=True,
                                         skip_group_check=True)
                        tile.add_dep_helper(m3.ins, m1.ins, sync=False,
                                            reason='hB after hA bank clear')
                        rT_hi = work.tile([128, ET], bfd, tag='rT_hi')
                        rT_lo = work.tile([16, ET], bfd, tag='rT_lo')
                        nc.scalar.activation(rT_hi[:], hA, AF.Relu, bias=fc1b_hi[:])
                        nc.scalar.activation(rT_lo[:], hB, AF.Relu, bias=fc1b_lo[:])

                        # fc2 -> w (SBUF): 1024-col PSUM groups, 512-col matmuls
                        wsb = work.tile([ET, CW], bfd, tag='wsb')
                        for g0 in range(0, CW, 1024):
                            g1 = min(g0 + 1024, CW)
                            wp = ps_w.tile([ET, 1024], fp, tag='wp')
                            for c0 in range(g0, g1, 512):
                                c1 = min(c0 + 512, g1)
                                nc.tensor.matmul(wp[:, c0 - g0:c1 - g0], rT_hi[:],
                                                 w2_hi[:, c0:c1],
                                                 start=True, stop=False)
                                nc.tensor.matmul(wp[:, c0 - g0:c1 - g0], rT_lo[:],
                                                 w2_lo[:, c0:c1],
                                                 start=False, stop=(c1 == g1))
                            if g0 == 3072 and CFG['tail_copy_dve']:
                                nc.vector.tensor_copy(wsb[:, g0:g1], wp[:, 0:g1 - g0])
                            else:
                                nc.scalar.copy(wsb[:, g0:g1], wp[:, 0:g1 - g0])

                        # TP products
                        prod_ss = work.tile([ET, NS * NS], bfd, tag='prod_ss')
                        nc.vector.tensor_tensor(
                            prod_ss[:].rearrange('p (o i) -> p o i', o=NS),
                            wsb[:, 0:OVS].rearrange('p (o i) -> p o i', o=NS),
                            xs1.unsqueeze(1).broadcast_to([ET, NS, NS]),
                            ALU.mult)
                        sp_eng = getattr(nc, CFG['small_prod_eng'])
                        cb_eng = getattr(nc, CFG['combine_eng'])
                        prod_vs = work.tile([ET, NS * NV], bfd, tag='prod_vs')
                        sp_eng.tensor_tensor(
                            prod_vs[:].rearrange('p (o i) -> p o i', o=NS),
                            wsb[:, OVS:OSV].rearrange('p (o i) -> p o i', o=NS),
                            dott.unsqueeze(1).broadcast_to([ET, NS, NV]),
                            ALU.mult)
                        prod_sv = work.tile([ET, NV * NS], bfd, tag='prod_sv')
                        sp_eng.tensor_tensor(
                            prod_sv[:].rearrange('p (o i) -> p o i', o=NV),
                            wsb[:, OSV:OVV].rearrange('p (o i) -> p o i', o=NV),
                            xst.unsqueeze(1).broadcast_to([ET, NV, NS]),
                            ALU.mult)
                        svt = work.tile([ET, NV], fp, tag='svt')
                        nc.vector.tensor_reduce(
                            svt[:], prod_sv[:].rearrange('p (o i) -> p o i', o=NV),
                            AX.X, ALU.add)
                        prod_vv = work.tile([ET, NV * 3 * NV], bfd, tag='prod_vv')
                        sp_eng.tensor_tensor(
                            prod_vv[:].rearrange('p (o c i) -> p o c i', o=NV, c=3),
                            wsb[:, OVV:CW].rearrange('p (o i) -> p o i', o=NV)
                                .unsqueeze(2).broadcast_to([ET, NV, 3, NV]),
                            xvc.rearrange('p (c i) -> p c i', c=3)
                                .unsqueeze(1).broadcast_to([ET, NV, 3, NV]),
                            ALU.mult)
                        u = work.tile([ET, NV * 3], fp, tag='u')
                        nc.vector.tensor_reduce(
                            u[:], prod_vv[:].rearrange('p (o c i) -> p o c i', o=NV, c=3),
                            AX.X, ALU.add)
                        tmp = work.tile([ET, NV * 3], bfd, tag='tmp')
                        cb_eng.tensor_tensor(
                            tmp[:].rearrange('p (o c) -> p o c', o=NV),
                            svt[:].unsqueeze(2).broadcast_to([ET, NV, 3]),
                            s1t.unsqueeze(1).broadcast_to([ET, NV, 3]),
                            ALU.mult)
                        cb_eng.tensor_tensor(tmp[:], tmp[:], u[:], ALU.add)
                        cb_eng.tensor_tensor(tpu[:, 48:78], tpu[:, 48:78],
                                                tmp[:], ALU.add)

                        # pre-reduce i halves (24 / 5 remain), then scatter
                        pss3 = prod_ss[:].rearrange('p (o i) -> p o i', o=NS)
                        if CFG['ss_pre']:
                            red_ss = work.tile([ET, NS * 24], bfd, tag='red_ss')
                            nc.vector.tensor_tensor(
                                red_ss[:].rearrange('p (o i) -> p o i', o=NS),
                                pss3[:, :, 0:24], pss3[:, :, 24:48], ALU.add)
                        else:
                            red_ss = prod_ss
                        if CFG['ss_quarters'] and CFG['ss_pre']:
                            red_ssq = work.tile([ET, NS * 12], bfd, tag='red_ssq')
                            rs3 = red_ss[:].rearrange('p (o i) -> p o i', o=NS)
                            nc.vector.tensor_tensor(
                                red_ssq[:].rearrange('p (o i) -> p o i', o=NS),
                                rs3[:, :, 0:12], rs3[:, :, 12:24], ALU.add)
                            nss, sstile = 12, red_ssq
                        else:
                            nss = 24 if CFG['ss_pre'] else NS
                            sstile = red_ss
                        psv3 = prod_vs[:].rearrange('p (o i) -> p o i', o=NS)
                        if CFG['vs_pre']:
                            red_vs = work.tile([ET, NS * 5], bfd, tag='red_vs')
                            nc.vector.tensor_tensor(
                                red_vs[:].rearrange('p (o i) -> p o i', o=NS),
                                psv3[:, :, 0:5], psv3[:, :, 5:10], ALU.add)
                            nvs = 5
                        else:
                            red_vs, nvs = prod_vs, NV
                        rss = sstile[:].rearrange('p (o i) -> p o i', o=NS)
                        rvs = red_vs[:].rearrange('p (o i) -> p o i', o=NS)
                        if CFG['two_bank']:
                            for i in range(nss):
                                tgt = acc[:, 0:48] if i % 2 == 0 else accb[:]
                                nc.tensor.matmul(tgt, oh, rss[:, :, i],
                                                 start=(first and i < 2),
                                                 stop=False)
                            for i in range(nvs):
                                tgt = acc[:, 0:48] if i % 2 == 0 else accb[:]
                                nc.tensor.matmul(tgt, oh, rvs[:, :, i],
                                                 start=False, stop=(last and i == nvs - 1 and False))
                            nc.tensor.matmul(acc[:], oh, tpu,
                                             start=False, stop=last)
                        else:
                            for i in range(nss):
                                nc.tensor.matmul(acc[:, 0:48], oh, rss[:, :, i],
                                                 start=(first and i == 0), stop=False)
                            for i in range(nvs):
                                nc.tensor.matmul(acc[:, 0:48], oh, rvs[:, :, i],
                                                 start=False, stop=False)
                            nc.tensor.matmul(acc[:], oh, tpu,
                                             start=False, stop=last)
                        if last:
                            if CFG['two_bank']:
                                nc.vector.tensor_copy(
                                    sums_sb[:, w * 79 + 48:(w + 1) * 79],
                                    acc[:, 48:79])
                                nc.vector.tensor_tensor(
                                    sums_sb[:, w * 79:w * 79 + 48],
                                    acc[:, 0:48], accb[:], ALU.add)
                            else:
                                nc.vector.tensor_copy(
                                    sums_sb[:, w * 79:(w + 1) * 79], acc[:])

            # ---------------- phase 2: nodes ----------------
            with tc.tile_pool(name='p2', bufs=2) as p2, \
                 tc.tile_pool(name='ps2', bufs=1, space='PSUM') as ps2, \
                 tc.tile_pool(name='ps2b', bufs=1, space='PSUM') as ps2b, \
                 tc.tile_pool(name='dram', bufs=1, space='DRAM') as dram:
                stat_ps = ps2.tile([1, 2 * IN_DIM], fp)
                for w in range(WPC):
                    cmax = p2.tile([128, 1], fp, tag='cmax')
                    nc.vector.tensor_scalar_max(
                        cmax[:], sums_sb[:, w * 79 + 78:w * 79 + 79], 1.0)
                    invc = p2.tile([128, 1], fp, tag='invc')
                    nc.vector.reciprocal(invc[:], cmax[:])
                    rs = resid_sb[:, w * IN_DIM:(w + 1) * IN_DIM]
                    nc.vector.scalar_tensor_tensor(
                        rs, sums_sb[:, w * 79:w * 79 + IN_DIM], invc[:],
                        na_sb[:, w * IN_DIM:(w + 1) * IN_DIM],
                        ALU.mult, ALU.add)
                    sq = p2.tile([128, IN_DIM], fp, tag='sq')
                    nc.scalar.square(sq[:], rs)
                    nc.tensor.matmul(stat_ps[:, 0:IN_DIM], ones128[:], rs,
                                     start=(w == 0), stop=False)
                    nc.tensor.matmul(stat_ps[:, IN_DIM:2 * IN_DIM], ones128[:],
                                     sq[:], start=False, stop=(w == WPC - 1))
                stat_sb = p2.tile([1, 2 * IN_DIM], fp, tag='stat_sb')
                nc.vector.tensor_copy(stat_sb[:], stat_ps[:])
                st_in = dram.tile([1, 2 * IN_DIM], fp)
                st_out = dram.tile([1, 2 * IN_DIM], fp)
                nc.gpsimd.dma_start(st_in[:], stat_sb[:])
                statr = p2.tile([1, 2 * IN_DIM], fp, tag='statr')
                if BENCH_NO_COLLECTIVE:
                    nc.gpsimd.dma_start(statr[:], st_in[:])
                else:
                    nc.gpsimd.collective_compute(
                        'AllReduce', ALU.add,
                        replica_groups=[list(range(NCORES))],
                        ins=[st_in.opt()], outs=[st_out.opt()])
                    nc.gpsimd.dma_start(statr[:], st_out[:])

                # finalize bn params (rows live on partition 0)
                invN = 1.0 / float(N_NODES)
                mu = p2.tile([1, NS], fp, tag='mu')
                nc.vector.tensor_scalar_mul(mu[:], statr[:, 0:NS], invN)
                ms = p2.tile([1, NS], fp, tag='ms')
                nc.vector.tensor_scalar_mul(ms[:], statr[:, IN_DIM:IN_DIM + NS], invN)
                var = p2.tile([1, NS], fp, tag='var')
                nc.vector.tensor_tensor(var[:], mu[:], mu[:], ALU.mult)
                nc.vector.tensor_tensor(var[:], ms[:], var[:], ALU.subtract)
                std = p2.tile([1, NS], fp, tag='std')
                nc.scalar.activation(std[:], var[:], AF.Sqrt, bias=epsc[:])
                istd = p2.tile([1, NS], fp, tag='istd')
                nc.vector.reciprocal(istd[:], std[:])
                scale_row = p2.tile([1, IN_DIM], fp, tag='scale_row')
                shift_row = p2.tile([1, IN_DIM], fp, tag='shift_row')
                nc.vector.tensor_tensor(scale_row[:, 0:NS], bnw_s[:], istd[:],
                                        ALU.mult)
                tmu = p2.tile([1, NS], fp, tag='tmu')
                nc.vector.tensor_tensor(tmu[:], mu[:], scale_row[:, 0:NS], ALU.mult)
                nc.vector.tensor_tensor(shift_row[:, 0:NS], bnb_s[:], tmu[:],
                                        ALU.subtract)
                fn = p2.tile([1, NV], fp, tag='fn')
                nc.vector.tensor_reduce(
                    fn[:], statr[:, IN_DIM + NS:2 * IN_DIM]
                        .rearrange('p (v c) -> p v c', v=NV),
                    AX.X, ALU.add)
                nc.vector.tensor_scalar_mul(fn[:], fn[:], invN / 3.0)
                sf = p2.tile([1, NV], fp, tag='sf')
                nc.scalar.activation(sf[:], fn[:], AF.Sqrt, bias=epsc[:])
                isf = p2.tile([1, NV], fp, tag='isf')
                nc.vector.reciprocal(isf[:], sf[:])
                scv = p2.tile([1, NV], fp, tag='scv')
                nc.vector.tensor_tensor(scv[:], bnw_v[:], isf[:], ALU.mult)
                nc.vector.tensor_copy(
                    scale_row[:, NS:IN_DIM].rearrange('p (v c) -> p v c', v=NV),
                    scv[:].unsqueeze(2).broadcast_to([1, NV, 3]))
                nc.vector.memset(shift_row[:, NS:IN_DIM], 0.0)

                bc_ps = ps2b.tile([128, 2 * IN_DIM], fp)
                nc.tensor.matmul(bc_ps[:, 0:IN_DIM], ones1[:], scale_row[:],
                                 start=True, stop=False)
                nc.tensor.matmul(bc_ps[:, IN_DIM:2 * IN_DIM], ones1[:],
                                 shift_row[:], start=False, stop=True)
                scale_bc = p2.tile([128, IN_DIM], fp, tag='scale_bc')
                shift_bc = p2.tile([128, IN_DIM], fp, tag='shift_bc')
                nc.vector.tensor_copy(scale_bc[:], bc_ps[:, 0:IN_DIM])
                nc.vector.tensor_copy(shift_bc[:], bc_ps[:, IN_DIM:2 * IN_DIM])
                for w in range(WPC):
                    ot = p2.tile([128, IN_DIM], fp, tag='ot')
                    nc.vector.tensor_tensor(
                        ot[:], resid_sb[:, w * IN_DIM:(w + 1) * IN_DIM],
                        scale_bc[:], ALU.mult)
                    nc.vector.tensor_tensor(ot[:], ot[:], shift_bc[:], ALU.add)
                    nc.sync.dma_start(d_out[w * 128:(w + 1) * 128, :], ot[:])

    nc.compile()
    return nc


# ----------------------------------------------------------------------------
# Entry point
# ----------------------------------------------------------------------------

def _make_in_maps(per_core, consts):
    in_maps = []
    for c in range(NCORES):
        pc = per_core[c]
        in_maps.append({
            'eaT': pc['eaT'], 'edata': pc['edata'], 'na': pc['na'],
            'fc1w': consts['fc1w'], 'fc1b': consts['fc1b'], 'w2p': consts['w2p'],
            'bnw_s': consts['bnw_s'], 'bnw_v': consts['bnw_v'],
            'bnb_s': consts['bnb_s'],
        })
    return in_maps


def kernel(**inputs):
    per_core, consts, t_list, (core_of_win, slot_of_win) = _prep(
        **{k: np.asarray(v) for k, v in inputs.items()})
    if t_list not in _cache:
        _cache[t_list] = _build(t_list)
    nc = _cache[t_list]
    in_maps = _make_in_maps(per_core, consts)
    res = bass_utils.run_bass_kernel_spmd(
        nc, in_maps, core_ids=list(range(NCORES)),
        trace=bool(int(os.environ.get('KERNEL_TRACE', '0'))))
    kernel.last_results = res
    kernel.last_nc = nc
    kernel.last_in_maps = in_maps
    out = np.empty((NCORES * CPN, IN_DIM), np.float32)
    for gw in range(NCORES * WPC):
        c, s = core_of_win[gw], slot_of_win[gw]
        out[gw * WIN:(gw + 1) * WIN] =             res.results[c]['out_shard'][s * WIN:(s + 1) * WIN]
    return out[:N_NODES].astype(np.float32)


# ----------------------------------------------------------------------------
# Execute-only timing helper (used by test.py, not by the grading harness)
# ----------------------------------------------------------------------------

def make_runner(nc, in_maps):
    """Build a cached PJRT executable + device-resident inputs; returns a
    zero-arg callable that executes the kernel once and blocks."""
    import jax
    from jax.experimental.shard_map import shard_map
    from jax.sharding import Mesh, PartitionSpec, NamedSharding
    from concourse import bass2jax, mybir as mb

    bass2jax.install_neuronx_cc_hook()
    partition_name = nc.partition_id_tensor.name if nc.partition_id_tensor else None
    in_names, out_names, out_avals = [], [], []
    for alloc in nc.m.functions[0].allocations:
        if not isinstance(alloc, mb.MemoryLocationSet):
            continue
        name = alloc.memorylocations[0].name
        if alloc.kind == 'ExternalInput':
            if name != partition_name:
                in_names.append(name)
        elif alloc.kind == 'ExternalOutput':
            out_names.append(name)
            out_avals.append(jax.core.ShapedArray(tuple(alloc.tensor_shape),
                                                  mb.dt.np(alloc.dtype)))
    n_params = len(in_names)
    all_in = list(in_names) + list(out_names)
    if partition_name is not None:
        all_in.append(partition_name)

    def _body(*args):
        operands = list(args)
        if partition_name is not None:
            operands.append(bass2jax.partition_id_tensor())
        outs = bass2jax._bass_exec_p.bind(
            *operands,
            out_avals=tuple(out_avals),
            in_names=tuple(all_in),
            out_names=tuple(out_names),
            lowering_input_output_aliases=(),
            sim_require_finite=True, sim_require_nnan=True, nc=nc)
        return tuple(outs)

    devices = jax.devices()[:NCORES]
    mesh = Mesh(np.asarray(devices), ('core',))
    nin = n_params + len(out_names)
    fn = jax.jit(shard_map(_body, mesh=mesh,
                           in_specs=(PartitionSpec('core'),) * nin,
                           out_specs=(PartitionSpec('core'),) * len(out_names),
                           check_rep=False))
    sh = NamedSharding(mesh, PartitionSpec('core'))
    args = [jax.device_put(
        np.concatenate([np.asarray(in_maps[c][n]) for c in range(NCORES)], axis=0),
        sh) for n in in_names]
    args += [jax.device_put(
        np.zeros((NCORES * a.shape[0], *a.shape[1:]), a.dtype), sh)
        for a in out_avals]

    def run():
        outs = fn(*args)
        jax.block_until_ready(outs)
        return outs
    return run

